# revision 8
# baseline (speedup 1.0000x reference)
"""MoE FFN (top-2 of 8 experts, SwiGLU) for 8 Trainium2 NeuronCores.

Strategy: expert parallelism with spill balancing. The router (tiny
[T,H]@[H,E] matmul + softmax + top-2) runs on host as part of sharding;
tokens are dispatched to the core owning their expert. Each core runs a dense
SwiGLU FFN over its gathered tokens in bf16 (fp32 PSUM accumulation), in a
feature-on-partition / token-on-free-dim layout so no on-device transposes are
needed and every weight byte is DMA'd exactly once, as a handful of large
contiguous transfers. The host applies the combine weights and scatter-adds
the per-expert outputs into the full output.

Load balancing: each core processes a fixed primary quota (QA = 2048 tokens,
4 full 512-token blocks) of its own expert, plus one small secondary block of
`qb` tokens holding spill from overloaded experts, with a second (duplicated)
weight stream for the donor expert. This keeps the SPMD program's padded
capacity at ~mean load instead of max load.

Per-core device program (two weight streams), with nht = H/128 h-tiles,
f-chunks of FCH columns (nft f-tiles each):
  g_T[f, t] = sum_i w1[h_i, f]^T @ x_T[h_i, t]        (PSUM accum over h-tiles)
  u_T[f, t] likewise with w2
  h_T[f, t] = silu(g_T + b1) * (u_T + b2)             (ACT + DVE, -> bf16)
  y_T[h, t] = sum_f w3[f, h]^T @ h_T[f, t] + b3       (PSUM accum per f-chunk,
                                                       accumulated in SBUF f32)
Weights stream through SBUF one f-chunk at a time; tokens/outputs are SBUF-
resident. Every matmul has a 128-row stationary operand in natural layout and
a [128, block] moving operand, so the PE runs back-to-back at stream rate.

Extra tricks vs the plain version:
 - PE warmup: a run of dummy matmuls on a memset tile issued before any
   data-dependent matmul, so the PE's p-state/HAM clock ramp burns during the
   initial DMA wait instead of during real work.
 - Fine-grained prologue DMAs: the first-needed pieces (x block-0 h-tile 0,
   w1 chunk-0 first f-tile columns) are issued first as small transfers on
   separate queues so the first real matmul can start ASAP.
 - Output DMAs are spread round-robin across 4 engine queues.
"""

import numpy as np
import ml_dtypes

E = 8       # experts == cores
K = 2       # top-k
H = 1024    # hidden
F = 4096    # ffn dim
BLK = 512   # max tokens per block (moving free dim of every matmul)
FCH = 512   # f-chunk size (weight streaming granularity); FCH % 128 == 0
QA = 2048   # primary per-core quota (4 full blocks)
WARM = 12   # warmup matmuls

NHT = H // 128    # h-tiles
NFCH = F // FCH   # f-chunks
NFT = FCH // 128  # f-tiles per chunk

_BF16 = ml_dtypes.bfloat16

_kernel_cache: dict[object, object] = {}
_last_in_maps = None


def _blocks_for(max_n: int):
    """Token-block sizes covering max_n tokens: full 512-blocks plus a small
    tail block, so padded capacity hugs the real max expert load."""
    max_n = max(max_n, 16)
    nfull, rem = divmod(max_n, BLK)
    rem = (rem + 15) // 16 * 16
    sizes = [BLK] * nfull + ([rem] if rem else [])
    blocks = []
    off = 0
    for sz in sizes:
        blocks.append((off, sz))
        off += sz
    return blocks, off


def _plan(loads):
    """Choose primary blocks + secondary spill block size, and pack spill
    chunks (expert, start-within-spill, len) into at most E chunks."""
    mx = max(loads)
    if mx <= QA:
        blocks, cap = _blocks_for(mx)
        return blocks, 0, []
    spill = {e: loads[e] - QA for e in range(E) if loads[e] > QA}
    tot = sum(spill.values())
    qb = max(16, (tot + E - 1) // E)
    qb = (qb + 15) // 16 * 16
    while True:
        chunks = []
        for e, s in sorted(spill.items()):
            n = (s + qb - 1) // qb
            chunks += [(e, i * qb, min(qb, s - i * qb)) for i in range(n)]
        if len(chunks) <= E:
            break
        qb += 16
    blocks = [(i * BLK, BLK) for i in range(QA // BLK)]
    return blocks, qb, chunks


def _build(blocks_p, qb, use_b2: bool):
    """Build the per-core Bass/Tile program."""
    import concourse.bass as bass  # noqa: F401
    import concourse.tile as tile
    from concourse import bacc, mybir

    bf16 = mybir.dt.bfloat16
    f32 = mybir.dt.float32
    AF = mybir.ActivationFunctionType

    use_sec = qb > 0
    blocks_s = [(QA, qb)] if use_sec else []
    blocks = list(blocks_p) + blocks_s
    cap = sum(sz for _, sz in blocks)

    nc = bacc.Bacc("TRN2", target_bir_lowering=False, debug=False, num_devices=E)

    # Host-side layouts are chosen so every DMA is a large 2D/3D transfer with
    # long contiguous rows (see kernel() for the packing).
    xT = nc.declare_dram_parameter("xT", [128, NHT * cap], bf16, isOutput=False)
    w1 = nc.declare_dram_parameter("w1", [NFCH, 128, NFT * H], bf16, isOutput=False)
    w2 = nc.declare_dram_parameter("w2", [NFCH, 128, NFT * H], bf16, isOutput=False)
    w3 = nc.declare_dram_parameter("w3", [NFCH, 128, NFT * H], bf16, isOutput=False)
    b1 = nc.declare_dram_parameter("b1", [128, F // 128], f32, isOutput=False)
    b3 = nc.declare_dram_parameter("b3", [128, NHT], f32, isOutput=False)
    if use_b2:
        b2 = nc.declare_dram_parameter("b2", [128, F // 128], f32, isOutput=False)
    if use_sec:
        w1s = nc.declare_dram_parameter("w1s", [NFCH, 128, NFT * H], bf16, isOutput=False)
        w2s = nc.declare_dram_parameter("w2s", [NFCH, 128, NFT * H], bf16, isOutput=False)
        w3s = nc.declare_dram_parameter("w3s", [NFCH, 128, NFT * H], bf16, isOutput=False)
        b1s = nc.declare_dram_parameter("b1s", [128, F // 128], f32, isOutput=False)
        b3s = nc.declare_dram_parameter("b3s", [128, NHT], f32, isOutput=False)
        if use_b2:
            b2s = nc.declare_dram_parameter("b2s", [128, F // 128], f32, isOutput=False)
    yT = nc.declare_dram_parameter("yT", [128, NHT * cap], f32, isOutput=True)

    with tile.TileContext(nc) as tc:
        with (
            tc.tile_pool(name="xp", bufs=1) as xp,
            tc.tile_pool(name="yp", bufs=1) as yp,
            tc.tile_pool(name="wp", bufs=2) as wp,
            tc.tile_pool(name="ws", bufs=1) as wsp,
            tc.tile_pool(name="hp", bufs=2) as hp,
            tc.tile_pool(name="sp", bufs=3) as sp,
            tc.tile_pool(name="bp", bufs=1) as bp,
            tc.tile_pool(name="pg", bufs=2, space="PSUM") as pg,
            tc.tile_pool(name="pu", bufs=2, space="PSUM") as pu,
            tc.tile_pool(name="py", bufs=2, space="PSUM") as py,
            tc.tile_pool(name="pw", bufs=1, space="PSUM") as pw,
        ):
            # --- PE warmup: burn the clock ramp during the DMA wait. ---
            wmt = bp.tile([128, 640], bf16, tag="wm", name="wmt")
            nc.vector.memset(wmt[:], 0.0)
            pwt = pw.tile([128, 512], f32, tag="pw", name="pwt")
            for _ in range(WARM):
                nc.tensor.matmul(pwt[:], wmt[:, 0:128], wmt[:, 128:640],
                                 start=True, stop=True)

            # Tokens (resident, bf16): one [128, NHT*cap] tile in BLOCK-major
            # column order — token block at offset `off` occupies columns
            # [NHT*off, NHT*(off+sz)), h-tile i contiguous inside it. The host
            # supplies the identical layout, so each block is ONE contiguous
            # 2D transfer with multi-KB rows.
            xall = xp.tile([128, NHT * cap], bf16, name="xall")

            def xsl(i, off, sz):  # moving operand [128, sz] for h-tile i
                base = NHT * off + i * sz
                return xall[:, base:base + sz]

            # Output accumulator (resident, f32), same column layout as xall.
            yall = yp.tile([128, NHT * cap], f32, name="yall")

            def ysl(i, off, sz):
                return yall[:, i * cap + off:i * cap + off + sz]

            # --- Prologue DMAs. Each engine DMA queue only keeps ~1-2
            # direct DMAs in flight (0.6us issue + completion handshake), but
            # a single big dma_start spreads across all 16 HW queue rows — so
            # issue FEW, LARGE transfers, one stream per queue, in
            # consumption order. The PE warmup covers the staging window.
            w1c = wp.tile([128, NFT * H], bf16, tag="w1", name="w1c")
            w2c = wp.tile([128, NFT * H], bf16, tag="w2", name="w2c")
            w3c = wp.tile([128, NFT * H], bf16, tag="w3", name="w3c")
            b1t = bp.tile([128, F // 128], f32, tag="b1", name="b1t")
            b3t = bp.tile([128, NHT], f32, tag="b3", name="b3t")
            nc.sync.dma_start(w1c[:], w1[0])
            nc.scalar.dma_start(xall[:, 0:NHT * 512], xT[:, 0:NHT * 512])
            nc.gpsimd.dma_start(b1t[:], b1[:])
            nc.gpsimd.dma_start(w2c[:], w2[0])
            nc.sync.dma_start(w3c[:], w3[0])
            nc.gpsimd.dma_start(b3t[:], b3[:])
            if use_b2:
                b2t = bp.tile([128, F // 128], f32, tag="b2", name="b2t")
                nc.gpsimd.dma_start(b2t[:], b2[:])
            # remaining primary token blocks, in consumption order
            for off, sz in blocks_p[1:]:
                lo, hi = NHT * off, NHT * (off + sz)
                nc.scalar.dma_start(xall[:, lo:hi], xT[:, lo:hi])
            # secondary stream: tokens, chunk-0 weights, biases
            if use_sec:
                lo, hi = NHT * QA, NHT * (QA + qb)
                nc.scalar.dma_start(xall[:, lo:hi], xT[:, lo:hi])
                w1sc = wsp.tile([128, NFT * H], bf16, tag="w1s", name="w1sc")
                w2sc = wsp.tile([128, NFT * H], bf16, tag="w2s", name="w2sc")
                w3sc = wsp.tile([128, NFT * H], bf16, tag="w3s", name="w3sc")
                nc.gpsimd.dma_start(w1sc[:], w1s[0])
                nc.gpsimd.dma_start(w2sc[:], w2s[0])
                nc.gpsimd.dma_start(w3sc[:], w3s[0])
                b1st = bp.tile([128, F // 128], f32, tag="b1s", name="b1st")
                nc.gpsimd.dma_start(b1st[:], b1s[:])
                b3st = bp.tile([128, NHT], f32, tag="b3s", name="b3st")
                nc.gpsimd.dma_start(b3st[:], b3s[:])
                if use_b2:
                    b2st = bp.tile([128, F // 128], f32, tag="b2s", name="b2st")
                    nc.gpsimd.dma_start(b2st[:], b2s[:])

            yq = [nc.sync, nc.scalar, nc.gpsimd]

            for fc in range(NFCH):
                if fc > 0:
                    # Stream this f-chunk's weights (each byte loaded once).
                    w1c = wp.tile([128, NFT * H], bf16, tag="w1", name="w1c")
                    nc.sync.dma_start(w1c[:], w1[fc])
                    w2c = wp.tile([128, NFT * H], bf16, tag="w2", name="w2c")
                    nc.scalar.dma_start(w2c[:], w2[fc])
                    w3c = wp.tile([128, NFT * H], bf16, tag="w3", name="w3c")
                    nc.sync.dma_start(w3c[:], w3[fc])
                    if use_sec:
                        w1sc = wsp.tile([128, NFT * H], bf16, tag="w1s", name="w1sc")
                        nc.gpsimd.dma_start(w1sc[:], w1s[fc])
                        w2sc = wsp.tile([128, NFT * H], bf16, tag="w2s", name="w2sc")
                        nc.gpsimd.dma_start(w2sc[:], w2s[fc])
                        w3sc = wsp.tile([128, NFT * H], bf16, tag="w3s", name="w3sc")
                        nc.gpsimd.dma_start(w3sc[:], w3s[fc])

                def stage_b(off, sz, ht_tiles, w3t, b3b):
                    # Stage B: y_T[h, tok] += w3_chunk.T @ h_T
                    # w3 columns: (j, h) -> f-tile j, output col h.
                    for i in range(NHT):
                        psy = py.tile([128, sz], f32, tag="y", name="psy")
                        for j in range(NFT):
                            nc.tensor.matmul(
                                psy[:],
                                w3t[:, j * H + i * 128:j * H + (i + 1) * 128],
                                ht_tiles[j][:],
                                start=(j == 0), stop=(j == NFT - 1),
                            )
                        if fc == 0:
                            nc.scalar.activation(
                                ysl(i, off, sz), psy[:], AF.Identity,
                                bias=b3b[:, i:i + 1],
                            )
                        else:
                            nc.vector.tensor_add(
                                ysl(i, off, sz), ysl(i, off, sz), psy[:]
                            )
                    if fc == NFCH - 1:
                        for i in range(NHT):
                            lo, hi = i * cap + off, i * cap + off + sz
                            yq[i % 3].dma_start(yT[:, lo:hi], yall[:, lo:hi])

                pending = None  # stage-B args awaiting execution
                for bi, (off, sz) in enumerate(blocks):
                    sec = use_sec and bi == len(blocks) - 1
                    cw1, cw2, cw3 = (w1sc, w2sc, w3sc) if sec else (w1c, w2c, w3c)
                    cb1 = b1st if sec else b1t
                    cb3 = b3st if sec else b3t
                    if use_b2:
                        cb2 = b2st if sec else b2t
                    # Stage A: h_T[f, tok] = silu(g_T + b1) * (u_T + b2)
                    # w1/w2 columns: (j, i, q) -> f-tile j, h-tile i, col q.
                    ht_tiles = []
                    for j in range(NFT):
                        fg = fc * NFT + j  # global f-tile index
                        psg = pg.tile([128, sz], f32, tag="g", name="psg")
                        for i in range(NHT):
                            base = (j * NHT + i) * 128
                            nc.tensor.matmul(
                                psg[:], cw1[:, base:base + 128], xsl(i, off, sz),
                                start=(i == 0), stop=(i == NHT - 1),
                            )
                        s = sp.tile([128, sz], f32, tag="s", name="stile")
                        nc.scalar.activation(
                            s[:], psg[:], AF.Silu, bias=cb1[:, fg:fg + 1]
                        )
                        psu = pu.tile([128, sz], f32, tag="u", name="psu")
                        for i in range(NHT):
                            base = (j * NHT + i) * 128
                            nc.tensor.matmul(
                                psu[:], cw2[:, base:base + 128], xsl(i, off, sz),
                                start=(i == 0), stop=(i == NHT - 1),
                            )
                        h = hp.tile([128, sz], bf16, tag=f"h{j}", name=f"htile{j}")
                        if use_b2:
                            u2 = sp.tile([128, sz], f32, tag="u2", name="u2tile")
                            nc.scalar.activation(
                                u2[:], psu[:], AF.Identity, bias=cb2[:, fg:fg + 1]
                            )
                            nc.vector.tensor_mul(h[:], s[:], u2[:])
                        else:
                            nc.vector.tensor_mul(h[:], s[:], psu[:])
                        ht_tiles.append(h)

                    if pending is not None:
                        stage_b(*pending)
                    pending = (off, sz, ht_tiles, cw3, cb3)
                stage_b(*pending)

    nc.finalize()
    return nc


def _route(x2d: np.ndarray, router_w: np.ndarray):
    """Host router: softmax over experts, top-2. Returns per-expert token
    index lists and combine weights."""
    logits = x2d @ router_w                       # [T, E]
    logits -= logits.max(axis=-1, keepdims=True)
    p = np.exp(logits, dtype=np.float32)
    p /= p.sum(axis=-1, keepdims=True)
    # top-2 expert ids per token (ties: lower index first, like lax.top_k)
    order = np.argsort(-p, axis=-1, kind="stable")[:, :K]   # [T, K]
    idx_e, cw_e = [], []
    for e in range(E):
        sel = np.nonzero((order == e).any(axis=1))[0]
        idx_e.append(sel)
        cw_e.append(p[sel, e])
    return idx_e, cw_e


def _pack_w12(w: np.ndarray) -> np.ndarray:
    """[H, F] f32 -> [NFCH, 128, NFT*NHT*128] bf16 with column order (j, i, q):
    chunk c, partition p, f-tile j, h-tile i, col q = w[i*128+p, c*FCH+j*128+q].
    """
    t = np.asarray(w, dtype=np.float32).reshape(NHT, 128, NFCH, NFT, 128)
    t = t.transpose(2, 1, 3, 0, 4)  # [c, p, j, i, q]
    return np.ascontiguousarray(t.astype(_BF16)).reshape(NFCH, 128, NFT * H)


def _pack_w3(w: np.ndarray) -> np.ndarray:
    """[F, H] f32 -> [NFCH, 128, NFT*H] bf16 with column order (j, h):
    chunk c, partition p (= f within f-tile j) -> w[c*FCH+j*128+p, h]."""
    t = np.asarray(w, dtype=np.float32).reshape(NFCH, NFT, 128, H)
    t = t.transpose(0, 2, 1, 3)  # [c, p, j, h]
    return np.ascontiguousarray(t.astype(_BF16)).reshape(NFCH, 128, NFT * H)


def _pack_xT(xg: np.ndarray, blocks) -> np.ndarray:
    """[cap, H] f32 -> [128, NHT*cap] bf16, block-major columns: block at
    token offset `off` spans cols [NHT*off, NHT*(off+sz)), h-tile i
    contiguous inside it: col = NHT*off + i*sz + t."""
    xb = xg.astype(_BF16)
    return np.ascontiguousarray(np.concatenate(
        [
            xb[off:off + sz].reshape(sz, NHT, 128)
            .transpose(2, 1, 0).reshape(128, NHT * sz)
            for off, sz in blocks
        ],
        axis=1,
    ))


def kernel(x, router_w, w1, b1, w2, b2, w3, b3):
    from concourse.bass_utils import run_bass_kernel_spmd

    B, S, _ = x.shape
    T = B * S
    x2d = np.ascontiguousarray(x, dtype=np.float32).reshape(T, H)

    idx_e, cw_e = _route(x2d, np.asarray(router_w, dtype=np.float32))
    loads = [len(i) for i in idx_e]
    blocks_p, qb, chunks = _plan(loads)
    use_sec = qb > 0
    blocks = blocks_p + ([(QA, qb)] if use_sec else [])
    cap = sum(sz for _, sz in blocks)
    qa = sum(sz for _, sz in blocks_p)

    use_b2 = bool(np.any(b2))
    key = (tuple(blocks_p), qb, use_b2)
    nc = _kernel_cache.get(key)
    if nc is None:
        nc = _build(blocks_p, qb, use_b2)
        _kernel_cache[key] = nc

    # pack each expert's weights once
    epack = {}
    for e in range(E):
        epack[e] = {
            "w1": _pack_w12(w1[e]),
            "w2": _pack_w12(w2[e]),
            "w3": _pack_w3(w3[e]),
            "b1": np.ascontiguousarray(
                np.asarray(b1[e], dtype=np.float32).reshape(F // 128, 128).T),
            "b3": np.ascontiguousarray(
                np.asarray(b3[e], dtype=np.float32).reshape(NHT, 128).T),
        }
        if use_b2:
            epack[e]["b2"] = np.ascontiguousarray(
                np.asarray(b2[e], dtype=np.float32).reshape(F // 128, 128).T)

    # chunk c -> core c; record (expert, token-index slice) per core
    sec_assign = [None] * E
    for c, (e, so, sl) in enumerate(chunks):
        sec_assign[c] = (e, idx_e[e][QA + so:QA + so + sl],
                         cw_e[e][QA + so:QA + so + sl])

    zw = np.zeros((NFCH, 128, NFT * H), dtype=_BF16) if use_sec else None
    zb1 = np.zeros((128, F // 128), dtype=np.float32) if use_sec else None
    zb3 = np.zeros((128, NHT), dtype=np.float32) if use_sec else None

    in_maps = []
    for e in range(E):
        idx = idx_e[e][:qa]
        xg = np.zeros((cap, H), dtype=np.float32)
        xg[: len(idx)] = x2d[idx]
        if use_sec and sec_assign[e] is not None:
            sidx = sec_assign[e][1]
            xg[QA:QA + len(sidx)] = x2d[sidx]
        m = {"xT": _pack_xT(xg, blocks)}
        m.update(epack[e])
        if use_sec:
            if sec_assign[e] is not None:
                dp = epack[sec_assign[e][0]]
                m.update({"w1s": dp["w1"], "w2s": dp["w2"], "w3s": dp["w3"],
                          "b1s": dp["b1"], "b3s": dp["b3"]})
                if use_b2:
                    m["b2s"] = dp["b2"]
            else:
                m.update({"w1s": zw, "w2s": zw, "w3s": zw,
                          "b1s": zb1, "b3s": zb3})
                if use_b2:
                    m["b2s"] = zb1
        in_maps.append(m)

    global _last_in_maps
    _last_in_maps = in_maps
    res = run_bass_kernel_spmd(nc, in_maps, core_ids=list(range(E)))

    out = np.zeros((T, H), dtype=np.float32)
    for e in range(E):
        # yT [128, NHT*cap] -> y[t, h]: y[t, i*128+p] = yT[p, i*cap+t]
        yTe = res.results[e]["yT"].reshape(128, NHT, cap)
        idx = idx_e[e][:qa]
        n = len(idx)
        ye = yTe[:, :, :n].transpose(2, 1, 0).reshape(n, H)
        out[idx] += ye * cw_e[e][:n, None]
        if use_sec and sec_assign[e] is not None:
            _, sidx, scw = sec_assign[e]
            ns = len(sidx)
            ys = yTe[:, :, QA:QA + ns].transpose(2, 1, 0).reshape(ns, H)
            out[sidx] += ys * scw[:, None]
    return out.reshape(B, S, H)


# revision 9
# speedup vs baseline: 1.0024x; 1.0024x over previous
"""MoE FFN (top-2 of 8 experts, SwiGLU) for 8 Trainium2 NeuronCores.

Strategy: expert parallelism with spill balancing. The router (tiny
[T,H]@[H,E] matmul + softmax + top-2) runs on host as part of sharding;
tokens are dispatched to the core owning their expert. Each core runs a dense
SwiGLU FFN over its gathered tokens in bf16 (fp32 PSUM accumulation), in a
feature-on-partition / token-on-free-dim layout so no on-device transposes are
needed and every weight byte is DMA'd exactly once, as a handful of large
contiguous transfers. The host applies the combine weights and scatter-adds
the per-expert outputs into the full output.

Load balancing: each core processes a fixed primary quota (QA = 2048 tokens,
4 full 512-token blocks) of its own expert, plus one small secondary block of
`qb` tokens holding spill from overloaded experts, with a second (duplicated)
weight stream for the donor expert. This keeps the SPMD program's padded
capacity at ~mean load instead of max load.

Per-core device program (two weight streams), with nht = H/128 h-tiles,
f-chunks of FCH columns (nft f-tiles each):
  g_T[f, t] = sum_i w1[h_i, f]^T @ x_T[h_i, t]        (PSUM accum over h-tiles)
  u_T[f, t] likewise with w2
  h_T[f, t] = silu(g_T + b1) * (u_T + b2)             (ACT + DVE, -> bf16)
  y_T[h, t] = sum_f w3[f, h]^T @ h_T[f, t] + b3       (PSUM accum per f-chunk,
                                                       accumulated in SBUF f32)
Weights stream through SBUF one f-chunk at a time; tokens/outputs are SBUF-
resident. Every matmul has a 128-row stationary operand in natural layout and
a [128, block] moving operand, so the PE runs back-to-back at stream rate.

Extra tricks vs the plain version:
 - PE warmup: a run of dummy matmuls on a memset tile issued before any
   data-dependent matmul, so the PE's p-state/HAM clock ramp burns during the
   initial DMA wait instead of during real work.
 - Fine-grained prologue DMAs: the first-needed pieces (x block-0 h-tile 0,
   w1 chunk-0 first f-tile columns) are issued first as small transfers on
   separate queues so the first real matmul can start ASAP.
 - Output DMAs are spread round-robin across 4 engine queues.
"""

import numpy as np
import ml_dtypes

E = 8       # experts == cores
K = 2       # top-k
H = 1024    # hidden
F = 4096    # ffn dim
BLK = 512   # max tokens per block (moving free dim of every matmul)
FCH = 512   # f-chunk size (weight streaming granularity); FCH % 128 == 0
QA = 2048   # primary per-core quota (4 full blocks)
WARM = 12   # warmup matmuls

NHT = H // 128    # h-tiles
NFCH = F // FCH   # f-chunks
NFT = FCH // 128  # f-tiles per chunk

_BF16 = ml_dtypes.bfloat16

_kernel_cache: dict[object, object] = {}
_last_in_maps = None


def _blocks_for(max_n: int):
    """Token-block sizes covering max_n tokens: full 512-blocks plus a small
    tail block, so padded capacity hugs the real max expert load."""
    max_n = max(max_n, 16)
    nfull, rem = divmod(max_n, BLK)
    rem = (rem + 15) // 16 * 16
    sizes = [BLK] * nfull + ([rem] if rem else [])
    blocks = []
    off = 0
    for sz in sizes:
        blocks.append((off, sz))
        off += sz
    return blocks, off


def _plan(loads):
    """Choose primary blocks + secondary spill block size, and pack spill
    chunks (expert, start-within-spill, len) into at most E chunks."""
    mx = max(loads)
    if mx <= QA:
        blocks, cap = _blocks_for(mx)
        return blocks, 0, []
    spill = {e: loads[e] - QA for e in range(E) if loads[e] > QA}
    tot = sum(spill.values())
    qb = max(16, (tot + E - 1) // E)
    qb = (qb + 15) // 16 * 16
    while True:
        chunks = []
        for e, s in sorted(spill.items()):
            n = (s + qb - 1) // qb
            chunks += [(e, i * qb, min(qb, s - i * qb)) for i in range(n)]
        if len(chunks) <= E:
            break
        qb += 16
    blocks = [(i * BLK, BLK) for i in range(QA // BLK)]
    return blocks, qb, chunks


def _build(blocks_p, qb, use_b2: bool):
    """Build the per-core Bass/Tile program."""
    import concourse.bass as bass  # noqa: F401
    import concourse.tile as tile
    from concourse import bacc, mybir

    bf16 = mybir.dt.bfloat16
    f32 = mybir.dt.float32
    AF = mybir.ActivationFunctionType

    use_sec = qb > 0
    blocks_s = [(QA, qb)] if use_sec else []
    blocks = list(blocks_p) + blocks_s
    cap = sum(sz for _, sz in blocks)

    nc = bacc.Bacc("TRN2", target_bir_lowering=False, debug=False, num_devices=E)

    # Host-side layouts are chosen so every DMA is a large 2D/3D transfer with
    # long contiguous rows (see kernel() for the packing).
    xT = nc.declare_dram_parameter("xT", [128, NHT * cap], bf16, isOutput=False)
    w1 = nc.declare_dram_parameter("w1", [NFCH, 128, NFT * H], bf16, isOutput=False)
    w2 = nc.declare_dram_parameter("w2", [NFCH, 128, NFT * H], bf16, isOutput=False)
    w3 = nc.declare_dram_parameter("w3", [NFCH, 128, NFT * H], bf16, isOutput=False)
    b1 = nc.declare_dram_parameter("b1", [128, F // 128], f32, isOutput=False)
    b3 = nc.declare_dram_parameter("b3", [128, NHT], f32, isOutput=False)
    if use_b2:
        b2 = nc.declare_dram_parameter("b2", [128, F // 128], f32, isOutput=False)
    if use_sec:
        w1s = nc.declare_dram_parameter("w1s", [NFCH, 128, NFT * H], bf16, isOutput=False)
        w2s = nc.declare_dram_parameter("w2s", [NFCH, 128, NFT * H], bf16, isOutput=False)
        w3s = nc.declare_dram_parameter("w3s", [NFCH, 128, NFT * H], bf16, isOutput=False)
        b1s = nc.declare_dram_parameter("b1s", [128, F // 128], f32, isOutput=False)
        b3s = nc.declare_dram_parameter("b3s", [128, NHT], f32, isOutput=False)
        if use_b2:
            b2s = nc.declare_dram_parameter("b2s", [128, F // 128], f32, isOutput=False)
    yT = nc.declare_dram_parameter("yT", [128, NHT * cap], f32, isOutput=True)

    with tile.TileContext(nc) as tc:
        with (
            tc.tile_pool(name="xp", bufs=1) as xp,
            tc.tile_pool(name="yp", bufs=1) as yp,
            tc.tile_pool(name="wp", bufs=2) as wp,
            tc.tile_pool(name="ws", bufs=1) as wsp,
            tc.tile_pool(name="hp", bufs=2) as hp,
            tc.tile_pool(name="sp", bufs=3) as sp,
            tc.tile_pool(name="bp", bufs=1) as bp,
            tc.tile_pool(name="pg", bufs=2, space="PSUM") as pg,
            tc.tile_pool(name="pu", bufs=2, space="PSUM") as pu,
            tc.tile_pool(name="py", bufs=2, space="PSUM") as py,
            tc.tile_pool(name="pw", bufs=1, space="PSUM") as pw,
        ):
            # --- PE warmup: burn the clock ramp during the DMA wait. ---
            wmt = bp.tile([128, 640], bf16, tag="wm", name="wmt")
            nc.vector.memset(wmt[:], 0.0)
            pwt = pw.tile([128, 512], f32, tag="pw", name="pwt")
            for _ in range(WARM):
                nc.tensor.matmul(pwt[:], wmt[:, 0:128], wmt[:, 128:640],
                                 start=True, stop=True)

            # Tokens (resident, bf16): one [128, NHT*cap] tile in BLOCK-major
            # column order — token block at offset `off` occupies columns
            # [NHT*off, NHT*(off+sz)), h-tile i contiguous inside it. The host
            # supplies the identical layout, so each block is ONE contiguous
            # 2D transfer with multi-KB rows.
            xall = xp.tile([128, NHT * cap], bf16, name="xall")

            def xsl(i, off, sz):  # moving operand [128, sz] for h-tile i
                base = NHT * off + i * sz
                return xall[:, base:base + sz]

            # Output accumulator (resident, f32), same column layout as xall.
            yall = yp.tile([128, NHT * cap], f32, name="yall")

            def ysl(i, off, sz):
                return yall[:, i * cap + off:i * cap + off + sz]

            # --- Prologue DMAs. Each engine DMA queue only keeps ~1-2
            # direct DMAs in flight (0.6us issue + completion handshake), but
            # a single big dma_start spreads across all 16 HW queue rows — so
            # issue FEW, LARGE transfers, one stream per queue, in
            # consumption order. The PE warmup covers the staging window.
            w1c = wp.tile([128, NFT * H], bf16, tag="w1", name="w1c")
            w2c = wp.tile([128, NFT * H], bf16, tag="w2", name="w2c")
            w3c = wp.tile([128, NFT * H], bf16, tag="w3", name="w3c")
            b1t = bp.tile([128, F // 128], f32, tag="b1", name="b1t")
            b3t = bp.tile([128, NHT], f32, tag="b3", name="b3t")
            nc.sync.dma_start(w1c[:, 0:2 * H], w1[0][:, 0:2 * H])
            nc.scalar.dma_start(xall[:, 0:2048], xT[:, 0:2048])
            nc.gpsimd.dma_start(b1t[:], b1[:])
            nc.sync.dma_start(w1c[:, 2 * H:NFT * H], w1[0][:, 2 * H:NFT * H])
            nc.scalar.dma_start(xall[:, 2048:NHT * 512], xT[:, 2048:NHT * 512])
            nc.gpsimd.dma_start(w2c[:, 0:2 * H], w2[0][:, 0:2 * H])
            nc.gpsimd.dma_start(w2c[:, 2 * H:NFT * H], w2[0][:, 2 * H:NFT * H])
            nc.sync.dma_start(w3c[:], w3[0])
            nc.gpsimd.dma_start(b3t[:], b3[:])
            if use_b2:
                b2t = bp.tile([128, F // 128], f32, tag="b2", name="b2t")
                nc.gpsimd.dma_start(b2t[:], b2[:])
            # remaining primary token blocks, in consumption order
            for off, sz in blocks_p[1:]:
                lo, hi = NHT * off, NHT * (off + sz)
                nc.scalar.dma_start(xall[:, lo:hi], xT[:, lo:hi])
            # secondary stream: tokens, chunk-0 weights, biases
            if use_sec:
                lo, hi = NHT * QA, NHT * (QA + qb)
                nc.scalar.dma_start(xall[:, lo:hi], xT[:, lo:hi])
                w1sc = wsp.tile([128, NFT * H], bf16, tag="w1s", name="w1sc")
                w2sc = wsp.tile([128, NFT * H], bf16, tag="w2s", name="w2sc")
                w3sc = wsp.tile([128, NFT * H], bf16, tag="w3s", name="w3sc")
                nc.gpsimd.dma_start(w1sc[:], w1s[0])
                nc.gpsimd.dma_start(w2sc[:], w2s[0])
                nc.gpsimd.dma_start(w3sc[:], w3s[0])
                b1st = bp.tile([128, F // 128], f32, tag="b1s", name="b1st")
                nc.gpsimd.dma_start(b1st[:], b1s[:])
                b3st = bp.tile([128, NHT], f32, tag="b3s", name="b3st")
                nc.gpsimd.dma_start(b3st[:], b3s[:])
                if use_b2:
                    b2st = bp.tile([128, F // 128], f32, tag="b2s", name="b2st")
                    nc.gpsimd.dma_start(b2st[:], b2s[:])

            yq = [nc.sync, nc.scalar, nc.gpsimd]

            for fc in range(NFCH):
                if fc > 0:
                    # Stream this f-chunk's weights (each byte loaded once).
                    w1c = wp.tile([128, NFT * H], bf16, tag="w1", name="w1c")
                    nc.sync.dma_start(w1c[:], w1[fc])
                    w2c = wp.tile([128, NFT * H], bf16, tag="w2", name="w2c")
                    nc.scalar.dma_start(w2c[:], w2[fc])
                    w3c = wp.tile([128, NFT * H], bf16, tag="w3", name="w3c")
                    nc.sync.dma_start(w3c[:], w3[fc])
                    if use_sec:
                        w1sc = wsp.tile([128, NFT * H], bf16, tag="w1s", name="w1sc")
                        nc.gpsimd.dma_start(w1sc[:], w1s[fc])
                        w2sc = wsp.tile([128, NFT * H], bf16, tag="w2s", name="w2sc")
                        nc.gpsimd.dma_start(w2sc[:], w2s[fc])
                        w3sc = wsp.tile([128, NFT * H], bf16, tag="w3s", name="w3sc")
                        nc.gpsimd.dma_start(w3sc[:], w3s[fc])

                def stage_b(off, sz, ht_tiles, w3t, b3b):
                    # Stage B: y_T[h, tok] += w3_chunk.T @ h_T
                    # w3 columns: (j, h) -> f-tile j, output col h.
                    for i in range(NHT):
                        psy = py.tile([128, sz], f32, tag="y", name="psy")
                        for j in range(NFT):
                            nc.tensor.matmul(
                                psy[:],
                                w3t[:, j * H + i * 128:j * H + (i + 1) * 128],
                                ht_tiles[j][:],
                                start=(j == 0), stop=(j == NFT - 1),
                            )
                        if fc == 0:
                            nc.scalar.activation(
                                ysl(i, off, sz), psy[:], AF.Identity,
                                bias=b3b[:, i:i + 1],
                            )
                        else:
                            nc.vector.tensor_add(
                                ysl(i, off, sz), ysl(i, off, sz), psy[:]
                            )
                    if fc == NFCH - 1:
                        for i in range(NHT):
                            lo, hi = i * cap + off, i * cap + off + sz
                            yq[i % 3].dma_start(yT[:, lo:hi], yall[:, lo:hi])

                pending = None  # stage-B args awaiting execution
                for bi, (off, sz) in enumerate(blocks):
                    sec = use_sec and bi == len(blocks) - 1
                    cw1, cw2, cw3 = (w1sc, w2sc, w3sc) if sec else (w1c, w2c, w3c)
                    cb1 = b1st if sec else b1t
                    cb3 = b3st if sec else b3t
                    if use_b2:
                        cb2 = b2st if sec else b2t
                    # Stage A: h_T[f, tok] = silu(g_T + b1) * (u_T + b2)
                    # w1/w2 columns: (j, i, q) -> f-tile j, h-tile i, col q.
                    ht_tiles = []
                    for j in range(NFT):
                        fg = fc * NFT + j  # global f-tile index
                        psg = pg.tile([128, sz], f32, tag="g", name="psg")
                        for i in range(NHT):
                            base = (j * NHT + i) * 128
                            nc.tensor.matmul(
                                psg[:], cw1[:, base:base + 128], xsl(i, off, sz),
                                start=(i == 0), stop=(i == NHT - 1),
                            )
                        s = sp.tile([128, sz], f32, tag="s", name="stile")
                        nc.scalar.activation(
                            s[:], psg[:], AF.Silu, bias=cb1[:, fg:fg + 1]
                        )
                        psu = pu.tile([128, sz], f32, tag="u", name="psu")
                        for i in range(NHT):
                            base = (j * NHT + i) * 128
                            nc.tensor.matmul(
                                psu[:], cw2[:, base:base + 128], xsl(i, off, sz),
                                start=(i == 0), stop=(i == NHT - 1),
                            )
                        h = hp.tile([128, sz], bf16, tag=f"h{j}", name=f"htile{j}")
                        if use_b2:
                            u2 = sp.tile([128, sz], f32, tag="u2", name="u2tile")
                            nc.scalar.activation(
                                u2[:], psu[:], AF.Identity, bias=cb2[:, fg:fg + 1]
                            )
                            nc.vector.tensor_mul(h[:], s[:], u2[:])
                        else:
                            nc.vector.tensor_mul(h[:], s[:], psu[:])
                        ht_tiles.append(h)

                    if pending is not None:
                        stage_b(*pending)
                    pending = (off, sz, ht_tiles, cw3, cb3)
                stage_b(*pending)

    nc.finalize()
    return nc


def _route(x2d: np.ndarray, router_w: np.ndarray):
    """Host router: softmax over experts, top-2. Returns per-expert token
    index lists and combine weights."""
    logits = x2d @ router_w                       # [T, E]
    logits -= logits.max(axis=-1, keepdims=True)
    p = np.exp(logits, dtype=np.float32)
    p /= p.sum(axis=-1, keepdims=True)
    # top-2 expert ids per token (ties: lower index first, like lax.top_k)
    order = np.argsort(-p, axis=-1, kind="stable")[:, :K]   # [T, K]
    idx_e, cw_e = [], []
    for e in range(E):
        sel = np.nonzero((order == e).any(axis=1))[0]
        idx_e.append(sel)
        cw_e.append(p[sel, e])
    return idx_e, cw_e


def _pack_w12(w: np.ndarray) -> np.ndarray:
    """[H, F] f32 -> [NFCH, 128, NFT*NHT*128] bf16 with column order (j, i, q):
    chunk c, partition p, f-tile j, h-tile i, col q = w[i*128+p, c*FCH+j*128+q].
    """
    t = np.asarray(w, dtype=np.float32).reshape(NHT, 128, NFCH, NFT, 128)
    t = t.transpose(2, 1, 3, 0, 4)  # [c, p, j, i, q]
    return np.ascontiguousarray(t.astype(_BF16)).reshape(NFCH, 128, NFT * H)


def _pack_w3(w: np.ndarray) -> np.ndarray:
    """[F, H] f32 -> [NFCH, 128, NFT*H] bf16 with column order (j, h):
    chunk c, partition p (= f within f-tile j) -> w[c*FCH+j*128+p, h]."""
    t = np.asarray(w, dtype=np.float32).reshape(NFCH, NFT, 128, H)
    t = t.transpose(0, 2, 1, 3)  # [c, p, j, h]
    return np.ascontiguousarray(t.astype(_BF16)).reshape(NFCH, 128, NFT * H)


def _pack_xT(xg: np.ndarray, blocks) -> np.ndarray:
    """[cap, H] f32 -> [128, NHT*cap] bf16, block-major columns: block at
    token offset `off` spans cols [NHT*off, NHT*(off+sz)), h-tile i
    contiguous inside it: col = NHT*off + i*sz + t."""
    xb = xg.astype(_BF16)
    return np.ascontiguousarray(np.concatenate(
        [
            xb[off:off + sz].reshape(sz, NHT, 128)
            .transpose(2, 1, 0).reshape(128, NHT * sz)
            for off, sz in blocks
        ],
        axis=1,
    ))


def kernel(x, router_w, w1, b1, w2, b2, w3, b3):
    from concourse.bass_utils import run_bass_kernel_spmd

    B, S, _ = x.shape
    T = B * S
    x2d = np.ascontiguousarray(x, dtype=np.float32).reshape(T, H)

    idx_e, cw_e = _route(x2d, np.asarray(router_w, dtype=np.float32))
    loads = [len(i) for i in idx_e]
    blocks_p, qb, chunks = _plan(loads)
    use_sec = qb > 0
    blocks = blocks_p + ([(QA, qb)] if use_sec else [])
    cap = sum(sz for _, sz in blocks)
    qa = sum(sz for _, sz in blocks_p)

    use_b2 = bool(np.any(b2))
    key = (tuple(blocks_p), qb, use_b2)
    nc = _kernel_cache.get(key)
    if nc is None:
        nc = _build(blocks_p, qb, use_b2)
        _kernel_cache[key] = nc

    # pack each expert's weights once
    epack = {}
    for e in range(E):
        epack[e] = {
            "w1": _pack_w12(w1[e]),
            "w2": _pack_w12(w2[e]),
            "w3": _pack_w3(w3[e]),
            "b1": np.ascontiguousarray(
                np.asarray(b1[e], dtype=np.float32).reshape(F // 128, 128).T),
            "b3": np.ascontiguousarray(
                np.asarray(b3[e], dtype=np.float32).reshape(NHT, 128).T),
        }
        if use_b2:
            epack[e]["b2"] = np.ascontiguousarray(
                np.asarray(b2[e], dtype=np.float32).reshape(F // 128, 128).T)

    # chunk c -> core c; record (expert, token-index slice) per core
    sec_assign = [None] * E
    for c, (e, so, sl) in enumerate(chunks):
        sec_assign[c] = (e, idx_e[e][QA + so:QA + so + sl],
                         cw_e[e][QA + so:QA + so + sl])

    zw = np.zeros((NFCH, 128, NFT * H), dtype=_BF16) if use_sec else None
    zb1 = np.zeros((128, F // 128), dtype=np.float32) if use_sec else None
    zb3 = np.zeros((128, NHT), dtype=np.float32) if use_sec else None

    in_maps = []
    for e in range(E):
        idx = idx_e[e][:qa]
        xg = np.zeros((cap, H), dtype=np.float32)
        xg[: len(idx)] = x2d[idx]
        if use_sec and sec_assign[e] is not None:
            sidx = sec_assign[e][1]
            xg[QA:QA + len(sidx)] = x2d[sidx]
        m = {"xT": _pack_xT(xg, blocks)}
        m.update(epack[e])
        if use_sec:
            if sec_assign[e] is not None:
                dp = epack[sec_assign[e][0]]
                m.update({"w1s": dp["w1"], "w2s": dp["w2"], "w3s": dp["w3"],
                          "b1s": dp["b1"], "b3s": dp["b3"]})
                if use_b2:
                    m["b2s"] = dp["b2"]
            else:
                m.update({"w1s": zw, "w2s": zw, "w3s": zw,
                          "b1s": zb1, "b3s": zb3})
                if use_b2:
                    m["b2s"] = zb1
        in_maps.append(m)

    global _last_in_maps
    _last_in_maps = in_maps
    res = run_bass_kernel_spmd(nc, in_maps, core_ids=list(range(E)))

    out = np.zeros((T, H), dtype=np.float32)
    for e in range(E):
        # yT [128, NHT*cap] -> y[t, h]: y[t, i*128+p] = yT[p, i*cap+t]
        yTe = res.results[e]["yT"].reshape(128, NHT, cap)
        idx = idx_e[e][:qa]
        n = len(idx)
        ye = yTe[:, :, :n].transpose(2, 1, 0).reshape(n, H)
        out[idx] += ye * cw_e[e][:n, None]
        if use_sec and sec_assign[e] is not None:
            _, sidx, scw = sec_assign[e]
            ns = len(sidx)
            ys = yTe[:, :, QA:QA + ns].transpose(2, 1, 0).reshape(ns, H)
            out[sidx] += ys * scw[:, None]
    return out.reshape(B, S, H)


# revision 12
# speedup vs baseline: 1.0028x; 1.0004x over previous
"""MoE FFN (top-2 of 8 experts, SwiGLU) for 8 Trainium2 NeuronCores.

Strategy: expert parallelism with spill balancing. The router (tiny
[T,H]@[H,E] matmul + softmax + top-2) runs on host as part of sharding;
tokens are dispatched to the core owning their expert. Each core runs a dense
SwiGLU FFN over its gathered tokens in bf16 (fp32 PSUM accumulation), in a
feature-on-partition / token-on-free-dim layout so no on-device transposes are
needed and every weight byte is DMA'd exactly once, as a handful of large
contiguous transfers. The host applies the combine weights and scatter-adds
the per-expert outputs into the full output.

Load balancing: each core processes a fixed primary quota (QA = 2048 tokens,
4 full 512-token blocks) of its own expert, plus one small secondary block of
`qb` tokens holding spill from overloaded experts, with a second (duplicated)
weight stream for the donor expert. This keeps the SPMD program's padded
capacity at ~mean load instead of max load.

Per-core device program (two weight streams), with nht = H/128 h-tiles,
f-chunks of FCH columns (nft f-tiles each):
  g_T[f, t] = sum_i w1[h_i, f]^T @ x_T[h_i, t]        (PSUM accum over h-tiles)
  u_T[f, t] likewise with w2
  h_T[f, t] = silu(g_T + b1) * (u_T + b2)             (ACT + DVE, -> bf16)
  y_T[h, t] = sum_f w3[f, h]^T @ h_T[f, t] + b3       (PSUM accum per f-chunk,
                                                       accumulated in SBUF f32)
Weights stream through SBUF one f-chunk at a time; tokens/outputs are SBUF-
resident. Every matmul has a 128-row stationary operand in natural layout and
a [128, block] moving operand, so the PE runs back-to-back at stream rate.

Extra tricks vs the plain version:
 - PE warmup: a run of dummy matmuls on a memset tile issued before any
   data-dependent matmul, so the PE's p-state/HAM clock ramp burns during the
   initial DMA wait instead of during real work.
 - Fine-grained prologue DMAs: the first-needed pieces (x block-0 h-tile 0,
   w1 chunk-0 first f-tile columns) are issued first as small transfers on
   separate queues so the first real matmul can start ASAP.
 - Output DMAs are spread round-robin across 4 engine queues.
"""

import numpy as np
import ml_dtypes

E = 8       # experts == cores
K = 2       # top-k
H = 1024    # hidden
F = 4096    # ffn dim
BLK = 512   # max tokens per block (moving free dim of every matmul)
FCH = 512   # f-chunk size (weight streaming granularity); FCH % 128 == 0
QA = 2048   # primary per-core quota (4 full blocks)
WARM = 24   # warmup matmuls

NHT = H // 128    # h-tiles
NFCH = F // FCH   # f-chunks
NFT = FCH // 128  # f-tiles per chunk

_BF16 = ml_dtypes.bfloat16

_kernel_cache: dict[object, object] = {}
_last_in_maps = None


def _blocks_for(max_n: int):
    """Token-block sizes covering max_n tokens: full 512-blocks plus a small
    tail block, so padded capacity hugs the real max expert load."""
    max_n = max(max_n, 16)
    nfull, rem = divmod(max_n, BLK)
    rem = (rem + 15) // 16 * 16
    sizes = [BLK] * nfull + ([rem] if rem else [])
    blocks = []
    off = 0
    for sz in sizes:
        blocks.append((off, sz))
        off += sz
    return blocks, off


def _plan(loads):
    """Choose primary blocks + secondary spill block size, and pack spill
    chunks (expert, start-within-spill, len) into at most E chunks."""
    mx = max(loads)
    if mx <= QA:
        blocks, cap = _blocks_for(mx)
        return blocks, 0, []
    spill = {e: loads[e] - QA for e in range(E) if loads[e] > QA}
    tot = sum(spill.values())
    qb = max(16, (tot + E - 1) // E)
    qb = (qb + 15) // 16 * 16
    while True:
        chunks = []
        for e, s in sorted(spill.items()):
            n = (s + qb - 1) // qb
            chunks += [(e, i * qb, min(qb, s - i * qb)) for i in range(n)]
        if len(chunks) <= E:
            break
        qb += 16
    blocks = [(i * BLK, BLK) for i in range(QA // BLK)]
    return blocks, qb, chunks


def _build(blocks_p, qb, use_b2: bool):
    """Build the per-core Bass/Tile program."""
    import concourse.bass as bass  # noqa: F401
    import concourse.tile as tile
    from concourse import bacc, mybir

    bf16 = mybir.dt.bfloat16
    f32 = mybir.dt.float32
    AF = mybir.ActivationFunctionType

    use_sec = qb > 0
    blocks_s = [(QA, qb)] if use_sec else []
    blocks = list(blocks_p) + blocks_s
    cap = sum(sz for _, sz in blocks)

    nc = bacc.Bacc("TRN2", target_bir_lowering=False, debug=False, num_devices=E)

    # Host-side layouts are chosen so every DMA is a large 2D/3D transfer with
    # long contiguous rows (see kernel() for the packing).
    xT = nc.declare_dram_parameter("xT", [128, NHT * cap], bf16, isOutput=False)
    w1 = nc.declare_dram_parameter("w1", [NFCH, 128, NFT * H], bf16, isOutput=False)
    w2 = nc.declare_dram_parameter("w2", [NFCH, 128, NFT * H], bf16, isOutput=False)
    w3 = nc.declare_dram_parameter("w3", [NFCH, 128, NFT * H], bf16, isOutput=False)
    b1 = nc.declare_dram_parameter("b1", [128, F // 128], f32, isOutput=False)
    b3 = nc.declare_dram_parameter("b3", [128, NHT], f32, isOutput=False)
    if use_b2:
        b2 = nc.declare_dram_parameter("b2", [128, F // 128], f32, isOutput=False)
    if use_sec:
        w1s = nc.declare_dram_parameter("w1s", [NFCH, 128, NFT * H], bf16, isOutput=False)
        w2s = nc.declare_dram_parameter("w2s", [NFCH, 128, NFT * H], bf16, isOutput=False)
        w3s = nc.declare_dram_parameter("w3s", [NFCH, 128, NFT * H], bf16, isOutput=False)
        b1s = nc.declare_dram_parameter("b1s", [128, F // 128], f32, isOutput=False)
        b3s = nc.declare_dram_parameter("b3s", [128, NHT], f32, isOutput=False)
        if use_b2:
            b2s = nc.declare_dram_parameter("b2s", [128, F // 128], f32, isOutput=False)
    yT = nc.declare_dram_parameter("yT", [128, NHT * cap], f32, isOutput=True)

    with tile.TileContext(nc) as tc:
        with (
            tc.tile_pool(name="xp", bufs=1) as xp,
            tc.tile_pool(name="yp", bufs=1) as yp,
            tc.tile_pool(name="wp", bufs=2) as wp,
            tc.tile_pool(name="ws", bufs=1) as wsp,
            tc.tile_pool(name="hp", bufs=2) as hp,
            tc.tile_pool(name="sp", bufs=3) as sp,
            tc.tile_pool(name="bp", bufs=1) as bp,
            tc.tile_pool(name="pg", bufs=2, space="PSUM") as pg,
            tc.tile_pool(name="pu", bufs=2, space="PSUM") as pu,
            tc.tile_pool(name="py", bufs=2, space="PSUM") as py,
            tc.tile_pool(name="pw", bufs=1, space="PSUM") as pw,
        ):
            # --- PE warmup: burn the clock ramp during the DMA wait. ---
            wmt = bp.tile([128, 640], bf16, tag="wm", name="wmt")
            nc.vector.memset(wmt[:], 0.0)
            pwt = pw.tile([128, 512], f32, tag="pw", name="pwt")
            for _ in range(WARM):
                nc.tensor.matmul(pwt[:], wmt[:, 0:128], wmt[:, 128:640],
                                 start=True, stop=True)

            # Tokens (resident, bf16): one [128, NHT*cap] tile in BLOCK-major
            # column order — token block at offset `off` occupies columns
            # [NHT*off, NHT*(off+sz)), h-tile i contiguous inside it. The host
            # supplies the identical layout, so each block is ONE contiguous
            # 2D transfer with multi-KB rows.
            xall = xp.tile([128, NHT * cap], bf16, name="xall")

            def xsl(i, off, sz):  # moving operand [128, sz] for h-tile i
                base = NHT * off + i * sz
                return xall[:, base:base + sz]

            # Output accumulator (resident, f32), same column layout as xall.
            yall = yp.tile([128, NHT * cap], f32, name="yall")

            def ysl(i, off, sz):
                return yall[:, i * cap + off:i * cap + off + sz]

            # --- Prologue DMAs. Each engine DMA queue only keeps ~1-2
            # direct DMAs in flight (0.6us issue + completion handshake), but
            # a single big dma_start spreads across all 16 HW queue rows — so
            # issue FEW, LARGE transfers, one stream per queue, in
            # consumption order. The PE warmup covers the staging window.
            w1c = wp.tile([128, NFT * H], bf16, tag="w1", name="w1c")
            w2c = wp.tile([128, NFT * H], bf16, tag="w2", name="w2c")
            w3c = wp.tile([128, NFT * H], bf16, tag="w3", name="w3c")
            b1t = bp.tile([128, F // 128], f32, tag="b1", name="b1t")
            b3t = bp.tile([128, NHT], f32, tag="b3", name="b3t")
            # Critical prefix, consumption order, j/h-granular so the first
            # matmuls can start as pieces land:
            nc.sync.dma_start(w1c[:, 0:H], w1[0][:, 0:H])
            nc.scalar.dma_start(xall[:, 0:1024], xT[:, 0:1024])
            nc.gpsimd.dma_start(b1t[:], b1[:])
            nc.sync.dma_start(w1c[:, H:2 * H], w1[0][:, H:2 * H])
            nc.scalar.dma_start(xall[:, 1024:2560], xT[:, 1024:2560])
            nc.gpsimd.dma_start(w2c[:, 0:2 * H], w2[0][:, 0:2 * H])
            nc.sync.dma_start(w1c[:, 2 * H:NFT * H], w1[0][:, 2 * H:NFT * H])
            nc.scalar.dma_start(xall[:, 2560:NHT * 512], xT[:, 2560:NHT * 512])
            nc.gpsimd.dma_start(w2c[:, 2 * H:NFT * H], w2[0][:, 2 * H:NFT * H])
            nc.sync.dma_start(w3c[:], w3[0])
            nc.gpsimd.dma_start(b3t[:], b3[:])
            if use_b2:
                b2t = bp.tile([128, F // 128], f32, tag="b2", name="b2t")
                nc.gpsimd.dma_start(b2t[:], b2[:])
            # Non-urgent streams are deferred so they don't steal HBM/queue
            # bandwidth from the critical prefix: secondary chunk-0 weights go
            # behind w3 on sync; x blocks 1..3 + secondary x are issued from
            # inside fc0's compute (interleaved into the scalar stream).
            if use_sec:
                w1sc = wsp.tile([128, NFT * H], bf16, tag="w1s", name="w1sc")
                w2sc = wsp.tile([128, NFT * H], bf16, tag="w2s", name="w2sc")
                w3sc = wsp.tile([128, NFT * H], bf16, tag="w3s", name="w3sc")
                nc.sync.dma_start(w1sc[:], w1s[0])
                nc.sync.dma_start(w2sc[:], w2s[0])
                nc.sync.dma_start(w3sc[:], w3s[0])
                b1st = bp.tile([128, F // 128], f32, tag="b1s", name="b1st")
                nc.gpsimd.dma_start(b1st[:], b1s[:])
                b3st = bp.tile([128, NHT], f32, tag="b3s", name="b3st")
                nc.gpsimd.dma_start(b3st[:], b3s[:])
                if use_b2:
                    b2st = bp.tile([128, F // 128], f32, tag="b2s", name="b2st")
                    nc.gpsimd.dma_start(b2st[:], b2s[:])

            # x ranges still to load, popped one per stage-A slot in fc0
            xq_pending = []
            for off, sz in blocks_p[1:]:
                lo, hi = NHT * off, NHT * (off + sz)
                mid = (lo + hi) // 2 // 512 * 512
                xq_pending += [(lo, mid), (mid, hi)]
            if use_sec:
                xq_pending.append((NHT * QA, NHT * (QA + qb)))

            yq = [nc.sync, nc.scalar, nc.gpsimd]

            for fc in range(NFCH):
                if fc > 0:
                    # Stream this f-chunk's weights (each byte loaded once).
                    w1c = wp.tile([128, NFT * H], bf16, tag="w1", name="w1c")
                    nc.sync.dma_start(w1c[:], w1[fc])
                    w2c = wp.tile([128, NFT * H], bf16, tag="w2", name="w2c")
                    nc.scalar.dma_start(w2c[:], w2[fc])
                    w3c = wp.tile([128, NFT * H], bf16, tag="w3", name="w3c")
                    nc.sync.dma_start(w3c[:], w3[fc])
                    if use_sec:
                        w1sc = wsp.tile([128, NFT * H], bf16, tag="w1s", name="w1sc")
                        nc.gpsimd.dma_start(w1sc[:], w1s[fc])
                        w2sc = wsp.tile([128, NFT * H], bf16, tag="w2s", name="w2sc")
                        nc.gpsimd.dma_start(w2sc[:], w2s[fc])
                        w3sc = wsp.tile([128, NFT * H], bf16, tag="w3s", name="w3sc")
                        nc.gpsimd.dma_start(w3sc[:], w3s[fc])

                def stage_b(off, sz, ht_tiles, w3t, b3b):
                    # Stage B: y_T[h, tok] += w3_chunk.T @ h_T
                    # w3 columns: (j, h) -> f-tile j, output col h.
                    for i in range(NHT):
                        psy = py.tile([128, sz], f32, tag="y", name="psy")
                        for j in range(NFT):
                            nc.tensor.matmul(
                                psy[:],
                                w3t[:, j * H + i * 128:j * H + (i + 1) * 128],
                                ht_tiles[j][:],
                                start=(j == 0), stop=(j == NFT - 1),
                            )
                        if fc == 0:
                            nc.scalar.activation(
                                ysl(i, off, sz), psy[:], AF.Identity,
                                bias=b3b[:, i:i + 1],
                            )
                        else:
                            nc.vector.tensor_add(
                                ysl(i, off, sz), ysl(i, off, sz), psy[:]
                            )
                    if fc == NFCH - 1:
                        for i in range(NHT):
                            lo, hi = i * cap + off, i * cap + off + sz
                            yq[i % 3].dma_start(yT[:, lo:hi], yall[:, lo:hi])

                pending = None  # stage-B args awaiting execution
                for bi, (off, sz) in enumerate(blocks):
                    sec = use_sec and bi == len(blocks) - 1
                    cw1, cw2, cw3 = (w1sc, w2sc, w3sc) if sec else (w1c, w2c, w3c)
                    cb1 = b1st if sec else b1t
                    cb3 = b3st if sec else b3t
                    if use_b2:
                        cb2 = b2st if sec else b2t
                    # Stage A: h_T[f, tok] = silu(g_T + b1) * (u_T + b2)
                    # w1/w2 columns: (j, i, q) -> f-tile j, h-tile i, col q.
                    ht_tiles = []
                    for j in range(NFT):
                        if fc == 0 and j >= 2 and xq_pending:
                            lo, hi = xq_pending.pop(0)
                            nc.scalar.dma_start(xall[:, lo:hi], xT[:, lo:hi])
                        fg = fc * NFT + j  # global f-tile index
                        psg = pg.tile([128, sz], f32, tag="g", name="psg")
                        for i in range(NHT):
                            base = (j * NHT + i) * 128
                            nc.tensor.matmul(
                                psg[:], cw1[:, base:base + 128], xsl(i, off, sz),
                                start=(i == 0), stop=(i == NHT - 1),
                            )
                        s = sp.tile([128, sz], f32, tag="s", name="stile")
                        nc.scalar.activation(
                            s[:], psg[:], AF.Silu, bias=cb1[:, fg:fg + 1]
                        )
                        psu = pu.tile([128, sz], f32, tag="u", name="psu")
                        for i in range(NHT):
                            base = (j * NHT + i) * 128
                            nc.tensor.matmul(
                                psu[:], cw2[:, base:base + 128], xsl(i, off, sz),
                                start=(i == 0), stop=(i == NHT - 1),
                            )
                        h = hp.tile([128, sz], bf16, tag=f"h{j}", name=f"htile{j}")
                        if use_b2:
                            u2 = sp.tile([128, sz], f32, tag="u2", name="u2tile")
                            nc.scalar.activation(
                                u2[:], psu[:], AF.Identity, bias=cb2[:, fg:fg + 1]
                            )
                            nc.vector.tensor_mul(h[:], s[:], u2[:])
                        else:
                            nc.vector.tensor_mul(h[:], s[:], psu[:])
                        ht_tiles.append(h)

                    if pending is not None:
                        stage_b(*pending)
                    pending = (off, sz, ht_tiles, cw3, cb3)
                stage_b(*pending)

    nc.finalize()
    return nc


def _route(x2d: np.ndarray, router_w: np.ndarray):
    """Host router: softmax over experts, top-2. Returns per-expert token
    index lists and combine weights."""
    logits = x2d @ router_w                       # [T, E]
    logits -= logits.max(axis=-1, keepdims=True)
    p = np.exp(logits, dtype=np.float32)
    p /= p.sum(axis=-1, keepdims=True)
    # top-2 expert ids per token (ties: lower index first, like lax.top_k)
    order = np.argsort(-p, axis=-1, kind="stable")[:, :K]   # [T, K]
    idx_e, cw_e = [], []
    for e in range(E):
        sel = np.nonzero((order == e).any(axis=1))[0]
        idx_e.append(sel)
        cw_e.append(p[sel, e])
    return idx_e, cw_e


def _pack_w12(w: np.ndarray) -> np.ndarray:
    """[H, F] f32 -> [NFCH, 128, NFT*NHT*128] bf16 with column order (j, i, q):
    chunk c, partition p, f-tile j, h-tile i, col q = w[i*128+p, c*FCH+j*128+q].
    """
    t = np.asarray(w, dtype=np.float32).reshape(NHT, 128, NFCH, NFT, 128)
    t = t.transpose(2, 1, 3, 0, 4)  # [c, p, j, i, q]
    return np.ascontiguousarray(t.astype(_BF16)).reshape(NFCH, 128, NFT * H)


def _pack_w3(w: np.ndarray) -> np.ndarray:
    """[F, H] f32 -> [NFCH, 128, NFT*H] bf16 with column order (j, h):
    chunk c, partition p (= f within f-tile j) -> w[c*FCH+j*128+p, h]."""
    t = np.asarray(w, dtype=np.float32).reshape(NFCH, NFT, 128, H)
    t = t.transpose(0, 2, 1, 3)  # [c, p, j, h]
    return np.ascontiguousarray(t.astype(_BF16)).reshape(NFCH, 128, NFT * H)


def _pack_xT(xg: np.ndarray, blocks) -> np.ndarray:
    """[cap, H] f32 -> [128, NHT*cap] bf16, block-major columns: block at
    token offset `off` spans cols [NHT*off, NHT*(off+sz)), h-tile i
    contiguous inside it: col = NHT*off + i*sz + t."""
    xb = xg.astype(_BF16)
    return np.ascontiguousarray(np.concatenate(
        [
            xb[off:off + sz].reshape(sz, NHT, 128)
            .transpose(2, 1, 0).reshape(128, NHT * sz)
            for off, sz in blocks
        ],
        axis=1,
    ))


def kernel(x, router_w, w1, b1, w2, b2, w3, b3):
    from concourse.bass_utils import run_bass_kernel_spmd

    B, S, _ = x.shape
    T = B * S
    x2d = np.ascontiguousarray(x, dtype=np.float32).reshape(T, H)

    idx_e, cw_e = _route(x2d, np.asarray(router_w, dtype=np.float32))
    loads = [len(i) for i in idx_e]
    blocks_p, qb, chunks = _plan(loads)
    use_sec = qb > 0
    blocks = blocks_p + ([(QA, qb)] if use_sec else [])
    cap = sum(sz for _, sz in blocks)
    qa = sum(sz for _, sz in blocks_p)

    use_b2 = bool(np.any(b2))
    key = (tuple(blocks_p), qb, use_b2)
    nc = _kernel_cache.get(key)
    if nc is None:
        nc = _build(blocks_p, qb, use_b2)
        _kernel_cache[key] = nc

    # pack each expert's weights once
    epack = {}
    for e in range(E):
        epack[e] = {
            "w1": _pack_w12(w1[e]),
            "w2": _pack_w12(w2[e]),
            "w3": _pack_w3(w3[e]),
            "b1": np.ascontiguousarray(
                np.asarray(b1[e], dtype=np.float32).reshape(F // 128, 128).T),
            "b3": np.ascontiguousarray(
                np.asarray(b3[e], dtype=np.float32).reshape(NHT, 128).T),
        }
        if use_b2:
            epack[e]["b2"] = np.ascontiguousarray(
                np.asarray(b2[e], dtype=np.float32).reshape(F // 128, 128).T)

    # chunk c -> core c; record (expert, token-index slice) per core
    sec_assign = [None] * E
    for c, (e, so, sl) in enumerate(chunks):
        sec_assign[c] = (e, idx_e[e][QA + so:QA + so + sl],
                         cw_e[e][QA + so:QA + so + sl])

    zw = np.zeros((NFCH, 128, NFT * H), dtype=_BF16) if use_sec else None
    zb1 = np.zeros((128, F // 128), dtype=np.float32) if use_sec else None
    zb3 = np.zeros((128, NHT), dtype=np.float32) if use_sec else None

    in_maps = []
    for e in range(E):
        idx = idx_e[e][:qa]
        xg = np.zeros((cap, H), dtype=np.float32)
        xg[: len(idx)] = x2d[idx]
        if use_sec and sec_assign[e] is not None:
            sidx = sec_assign[e][1]
            xg[QA:QA + len(sidx)] = x2d[sidx]
        m = {"xT": _pack_xT(xg, blocks)}
        m.update(epack[e])
        if use_sec:
            if sec_assign[e] is not None:
                dp = epack[sec_assign[e][0]]
                m.update({"w1s": dp["w1"], "w2s": dp["w2"], "w3s": dp["w3"],
                          "b1s": dp["b1"], "b3s": dp["b3"]})
                if use_b2:
                    m["b2s"] = dp["b2"]
            else:
                m.update({"w1s": zw, "w2s": zw, "w3s": zw,
                          "b1s": zb1, "b3s": zb3})
                if use_b2:
                    m["b2s"] = zb1
        in_maps.append(m)

    global _last_in_maps
    _last_in_maps = in_maps
    res = run_bass_kernel_spmd(nc, in_maps, core_ids=list(range(E)))

    out = np.zeros((T, H), dtype=np.float32)
    for e in range(E):
        # yT [128, NHT*cap] -> y[t, h]: y[t, i*128+p] = yT[p, i*cap+t]
        yTe = res.results[e]["yT"].reshape(128, NHT, cap)
        idx = idx_e[e][:qa]
        n = len(idx)
        ye = yTe[:, :, :n].transpose(2, 1, 0).reshape(n, H)
        out[idx] += ye * cw_e[e][:n, None]
        if use_sec and sec_assign[e] is not None:
            _, sidx, scw = sec_assign[e]
            ns = len(sidx)
            ys = yTe[:, :, QA:QA + ns].transpose(2, 1, 0).reshape(ns, H)
            out[sidx] += ys * scw[:, None]
    return out.reshape(B, S, H)
